# revision 1
# baseline (speedup 1.0000x reference)
"""MoE top-1 routing kernel for Trainium2 (8 NeuronCores, expert-parallel).

Model (E=8, D=512, F=2048, N=4096):
    logits = x @ Wg + bg; e = argmax(logits)
    y[i] = relu(x[i] @ W1[e] + b1[e]) @ W2[e] + b2[e]

Strategy: the tiny gate + argmax + token routing run on the host (f64 gate
matmul, numerically dominating the reference's f32 argmax). Core e receives
only the tokens routed to expert e (padded to the max expert count C) plus
expert e's weights, and runs a dense 2-layer MLP in fp32r (full-rate fp32
matmul mode, ~1e-4 relative error). Host scatters per-core outputs back.

Token axis is the matmul free (streaming) dim, so C needs no 128 padding;
tokens are processed in chunks of <=512 columns (PSUM bank limit), >=256
wide where possible (fp32r full-rate threshold).
"""

import sys

sys.path.insert(0, "/opt/trn_rl_repo")

import numpy as np

E, D, F, N_CORES = 8, 512, 2048, 8
KD, KF = D // 128, F // 128  # 4, 16

_cache: dict = {}


def _build(C: int, chunks: list[tuple[int, int]]):
    import concourse.tile as tile
    import concourse.mybir as mybir
    from concourse import bacc

    f32, f32r = mybir.dt.float32, mybir.dt.float32r
    Relu = mybir.ActivationFunctionType.Relu

    nc = bacc.Bacc("TRN2", target_bir_lowering=False, debug=False)
    xT = nc.dram_tensor("xT", [D, C], f32r, kind="ExternalInput").ap()
    w1 = nc.dram_tensor("w1", [D, F], f32r, kind="ExternalInput").ap()
    b1 = nc.dram_tensor("b1", [F], f32, kind="ExternalInput").ap()
    w2 = nc.dram_tensor("w2", [F, D], f32r, kind="ExternalInput").ap()
    b2 = nc.dram_tensor("b2", [D], f32, kind="ExternalInput").ap()
    yT = nc.dram_tensor("yT", [D, C], f32, kind="ExternalOutput").ap()

    x3 = xT.rearrange("(ko ki) c -> ko ki c", ki=128)
    w13 = w1.rearrange("(ko ki) f -> ko ki f", ki=128)
    w23 = w2.rearrange("(fo fi) d -> fo fi d", fi=128)
    b1v = b1.rearrange("(fo fi) -> fi fo", fi=128)
    b2v = b2.rearrange("(do di) -> di do", di=128)
    y3 = yT.rearrange("(do di) c -> do di c", di=128)

    with tile.TileContext(nc) as tc:
        with tc.tile_pool(name="wp", bufs=1) as wp, \
             tc.tile_pool(name="xp", bufs=2) as xp, \
             tc.tile_pool(name="hp", bufs=2) as hp, \
             tc.tile_pool(name="yp", bufs=3) as yp, \
             tc.tile_pool(name="pp", bufs=3, space="PSUM") as pp:
            # weights + biases, loaded once
            w1s = [wp.tile([128, F], f32r, name=f"w1s{k}", tag=f"w1s{k}")
                   for k in range(KD)]
            for k in range(KD):
                # split the load so early f-tiles unblock the first matmuls
                for g in range(4):
                    nc.sync.dma_start(w1s[k][:, g * 512:(g + 1) * 512],
                                      w13[k][:, g * 512:(g + 1) * 512])
            w2s = [wp.tile([128, D], f32r, name=f"w2s{f}", tag=f"w2s{f}")
                   for f in range(KF)]
            for f in range(KF):
                nc.sync.dma_start(w2s[f][:], w23[f])
            b1s = wp.tile([128, KF], f32, name="b1s")
            b2s = wp.tile([128, KD], f32, name="b2s")
            nc.sync.dma_start(b1s[:], b1v)
            nc.sync.dma_start(b2s[:], b2v)

            for c0, c1 in chunks:
                cw = c1 - c0
                xs = [xp.tile([128, cw], f32r, name=f"xs{k}", tag=f"xs{k}")
                      for k in range(KD)]
                for k in range(KD):
                    nc.sync.dma_start(xs[k][:], x3[k][:, c0:c1])
                hs = []
                for f in range(KF):
                    p1 = pp.tile([128, cw], f32, name=f"p1_{f}", tag="p1")
                    for k in range(KD):
                        nc.tensor.matmul(p1[:], w1s[k][:, f * 128:(f + 1) * 128],
                                         xs[k][:], start=(k == 0), stop=(k == KD - 1))
                    h = hp.tile([128, cw], f32r, name=f"h{f}", tag=f"h{f}")
                    nc.scalar.activation(h[:], p1[:], Relu, bias=b1s[:, f:f + 1])
                    hs.append(h)
                for d in range(KD):
                    p2 = pp.tile([128, cw], f32, name=f"p2_{d}", tag="p2")
                    for f in range(KF):
                        nc.tensor.matmul(p2[:], w2s[f][:, d * 128:(d + 1) * 128],
                                         hs[f][:], start=(f == 0), stop=(f == KF - 1))
                    yt = yp.tile([128, cw], f32, name=f"y{d}", tag="y")
                    nc.vector.tensor_scalar_add(yt[:], p2[:], b2s[:, d:d + 1])
                    nc.sync.dma_start(y3[d][:, c0:c1], yt[:])
    nc.compile()
    return nc


def _plan_chunks(C: int) -> list[tuple[int, int]]:
    n = max(1, -(-C // 512))
    base, rem = divmod(C, n)
    out, pos = [], 0
    for i in range(n):
        w = base + (1 if i < rem else 0)
        out.append((pos, pos + w))
        pos += w
    return out


def _get_nc(C: int):
    key = C
    if key not in _cache:
        _cache[key] = _build(C, _plan_chunks(C))
    return _cache[key]


def kernel(x, Wg, bg, W1, b1, W2, b2):
    from concourse.bass_utils import run_bass_kernel_spmd

    x = np.asarray(x, dtype=np.float32)
    n_tok = x.shape[0]

    # host gate in f64: the mathematically-true argmax
    logits = x.astype(np.float64) @ np.asarray(Wg, np.float64) + np.asarray(bg, np.float64)
    idx = logits.argmax(1)

    counts = np.bincount(idx, minlength=E)
    order = np.argsort(idx, kind="stable")
    starts = np.zeros(E + 1, np.int64)
    starts[1:] = np.cumsum(counts)

    C = max(int(counts.max()), 256)
    C = (C + 15) // 16 * 16

    W1 = np.asarray(W1, np.float32)
    W2 = np.asarray(W2, np.float32)
    b1 = np.asarray(b1, np.float32)
    b2 = np.asarray(b2, np.float32)

    in_maps, toks_per_core = [], []
    for e in range(E):
        toks = order[starts[e]:starts[e + 1]]
        toks_per_core.append(toks)
        xe = np.zeros((C, D), np.float32)
        xe[:len(toks)] = x[toks]
        in_maps.append({
            "xT": np.ascontiguousarray(xe.T),
            "w1": np.ascontiguousarray(W1[e]),
            "b1": np.ascontiguousarray(b1[e]),
            "w2": np.ascontiguousarray(W2[e]),
            "b2": np.ascontiguousarray(b2[e]),
        })

    nc = _get_nc(C)
    res = run_bass_kernel_spmd(nc, in_maps, core_ids=list(range(N_CORES)))

    out = np.zeros((n_tok, D), np.float32)
    for e in range(E):
        toks = toks_per_core[e]
        out[toks] = res.results[e]["yT"][:, :len(toks)].T
    return out


# revision 4
# speedup vs baseline: 1.4324x; 1.4324x over previous
"""MoE top-1 routing kernel for Trainium2 (8 NeuronCores, expert-parallel).

Model (E=8, D=512, F=2048, N=4096):
    logits = x @ Wg + bg; e = argmax(logits)
    y[i] = relu(x[i] @ W1[e] + b1[e]) @ W2[e] + b2[e]

Strategy:
- Host computes the gate (f64 matmul + argmax) and routes tokens; core e gets
  only expert e's tokens (padded to the max expert count C) + expert e's
  weights, and runs a dense 2-layer MLP in fp32r (full-rate fp32 matmul mode,
  ~2e-4 scale-relative error).
- All tensors are host-packed into SBUF-native [128, *] layouts so every DMA
  moves multi-KB contiguous runs per partition (one dma_start per piece).
- DMA pieces are issued in consumption order (x chunk 0, W1 by f-groups,
  W2 by fo-groups) so matmuls start ~5us in; stage-2 accumulation is emitted
  in W2-piece arrival order, interleaved across PSUM banks.
- A short dummy-matmul burst warms the PE clock (HAM) during the DMA head.
- Tokens ride the matmul free dim in chunks of <=512 columns (PSUM bank
  limit), >=256 wide where possible (fp32r full-rate threshold).
"""

import sys

sys.path.insert(0, "/opt/trn_rl_repo")

import numpy as np

E, D, F, N_CORES = 8, 512, 2048, 8
KD, KF = D // 128, F // 128  # 4, 16
G1, G2 = KF // 4, KF // 4    # w1 f-piece count, w2 fo-piece count (4 each)

_cache: dict = {}


def _build(C: int, chunks: list[tuple[int, int]]):
    import concourse.tile as tile
    import concourse.mybir as mybir
    from concourse import bacc

    f32, f32r = mybir.dt.float32, mybir.dt.float32r
    Relu = mybir.ActivationFunctionType.Relu

    nc = bacc.Bacc("TRN2", target_bir_lowering=False, debug=False)
    # packed layouts, all [128, *]:
    #   xTi[p, chunk_off + ko*cw + c] = x_e[c0+c, 128*ko+p]
    #   w1i[p, g*2048 + ko*512 + fi]  = W1_e[128*ko+p, 512*g+fi]
    #   w2i[p, h*2048 + j*512 + d]    = W2_e[128*(4h+j)+p, d]
    #   bi[p, f] = b1_e[128f+p] (f<16);  bi[p, 16+d] = b2_e[128d+p]
    #   yTi[p, d*C + c] = y_e[c, 128d+p]
    xTi = nc.dram_tensor("xTi", [128, KD * C], f32r, kind="ExternalInput").ap()
    w1i = nc.dram_tensor("w1i", [128, KD * F], f32r, kind="ExternalInput").ap()
    w2i = nc.dram_tensor("w2i", [128, KF * D], f32r, kind="ExternalInput").ap()
    bi = nc.dram_tensor("bi", [128, KF + KD], f32, kind="ExternalInput").ap()
    yTi = nc.dram_tensor("yTi", [128, KD * C], f32, kind="ExternalOutput").ap()
    y3 = yTi.rearrange("p (d c) -> p d c", c=C)

    with tile.TileContext(nc) as tc:
        with tc.tile_pool(name="wp", bufs=1) as wp, \
             tc.tile_pool(name="hp", bufs=1) as hp, \
             tc.tile_pool(name="yp", bufs=2) as yp, \
             tc.tile_pool(name="scr", bufs=1) as scr, \
             tc.tile_pool(name="pp", bufs=3, space="PSUM") as pp:

            # --- PE warm-up: dummy matmuls during the DMA head (HAM ramp).
            # f32 runs 4 cycles/row, so 4x N=256 covers the ~3.4us window.
            wrm = scr.tile([128, 256], f32, name="wrm")
            nc.gpsimd.memset(wrm[:], 0.0)
            wps = pp.tile([128, 256], f32, name="wps", tag="wps", bufs=1)
            for _ in range(5):
                nc.tensor.matmul(wps[:], wrm[:, :128], wrm[:], start=True, stop=True)

            # --- DMA issue, consumption order, single engine (sync) ---
            bis = wp.tile([128, KF + KD], f32, name="bis")
            nc.sync.dma_start(bis[:], bi[:])

            w1t = wp.tile([128, KD * F], f32r, name="w1t")
            w2t = wp.tile([128, KF * D], f32r, name="w2t")
            xs = []
            off = 0
            for ci, (c0, c1) in enumerate(chunks):
                cw = c1 - c0
                xst = wp.tile([128, KD * cw], f32r, name=f"xs{ci}", tag=f"xs{ci}")
                xs.append((xst, off))
                off += KD * cw
            # order: x_c0, w1g0, x_c1, w1g1..g3, x_c2.., w2h0..h3
            nc.sync.dma_start(xs[0][0][:], xTi[:, xs[0][1]:xs[0][1] + xs[0][0].shape[-1]])
            nc.sync.dma_start(w1t[:, 0:2048], w1i[:, 0:2048])
            if len(xs) > 1:
                nc.sync.dma_start(xs[1][0][:], xTi[:, xs[1][1]:xs[1][1] + xs[1][0].shape[-1]])
            for g in range(1, G1):
                nc.sync.dma_start(w1t[:, g * 2048:(g + 1) * 2048],
                                  w1i[:, g * 2048:(g + 1) * 2048])
            for xst, o in xs[2:]:
                nc.sync.dma_start(xst[:], xTi[:, o:o + xst.shape[-1]])
            for h in range(G2):
                nc.sync.dma_start(w2t[:, h * 2048:(h + 1) * 2048],
                                  w2i[:, h * 2048:(h + 1) * 2048])

            # --- stage 1: h = relu(x @ W1 + b1), emitted in w1-piece order ---
            hs = {}  # (ci, f) -> tile
            for g in range(G1):
                for ci, (c0, c1) in enumerate(chunks):
                    cw = c1 - c0
                    xst = xs[ci][0]
                    for f in range(4 * g, 4 * g + 4):
                        p1 = pp.tile([128, cw], f32, name=f"p1_{ci}_{f}", tag="p1")
                        for ko in range(KD):
                            lhsT = w1t[:, g * 2048 + ko * 512 + (f % 4) * 128:
                                       g * 2048 + ko * 512 + (f % 4) * 128 + 128]
                            nc.tensor.matmul(p1[:], lhsT, xst[:, ko * cw:(ko + 1) * cw],
                                             start=(ko == 0), stop=(ko == KD - 1))
                        h = hp.tile([128, cw], f32r, name=f"h{ci}_{f}",
                                    tag=f"h{ci % 2}_{f}")
                        nc.scalar.activation(h[:], p1[:], Relu, bias=bis[:, f:f + 1])
                        hs[(ci, f)] = h

            # --- stage 2: y = h @ W2 + b2, emitted in w2-piece arrival order ---
            for ci, (c0, c1) in enumerate(chunks):
                cw = c1 - c0
                p2s = [pp.tile([128, cw], f32, name=f"p2_{ci}_{d}", tag=f"p2_{d}",
                               bufs=1) for d in range(KD)]
                for h2 in range(G2):
                    for d in range(KD):
                        for j in range(4):
                            fo = 4 * h2 + j
                            lhsT = w2t[:, h2 * 2048 + j * 512 + d * 128:
                                       h2 * 2048 + j * 512 + d * 128 + 128]
                            nc.tensor.matmul(p2s[d][:], lhsT, hs[(ci, fo)][:],
                                             start=(fo == 0), stop=(fo == KF - 1))
                ys = yp.tile([128, KD, cw], f32, name=f"ys{ci}", tag="ys")
                for d in range(KD):
                    nc.vector.tensor_scalar_add(ys[:, d, :], p2s[d][:],
                                                bis[:, KF + d:KF + d + 1])
                nc.gpsimd.dma_start(y3[:, :, c0:c1], ys[:])
    nc.compile()
    return nc


def _plan_chunks(C: int) -> list[tuple[int, int]]:
    n = max(1, -(-C // 512))
    base, rem = divmod(C, n)
    out, pos = [], 0
    for i in range(n):
        w = base + (1 if i < rem else 0)
        out.append((pos, pos + w))
        pos += w
    return out


def _get_nc(C: int):
    if C not in _cache:
        _cache[C] = _build(C, _plan_chunks(C))
    return _cache[C]


def _pack_inputs(x, W1, b1, W2, b2, idx, order, starts, C):
    chunks = _plan_chunks(C)
    in_maps, toks_per_core = [], []
    for e in range(E):
        toks = order[starts[e]:starts[e + 1]]
        toks_per_core.append(toks)
        xe = np.zeros((C, D), np.float32)
        xe[:len(toks)] = x[toks]
        xeT = xe.T  # [D, C]
        xTi = np.concatenate(
            [xeT[:, c0:c1].reshape(KD, 128, c1 - c0).transpose(1, 0, 2)
             .reshape(128, KD * (c1 - c0)) for c0, c1 in chunks], axis=1)
        w1p = np.concatenate(
            [W1[e][:, 512 * g:512 * (g + 1)].reshape(KD, 128, 512)
             .transpose(1, 0, 2).reshape(128, KD * 512) for g in range(G1)], axis=1)
        w2p = np.concatenate(
            [W2[e][512 * h:512 * (h + 1), :].reshape(4, 128, 512)
             .transpose(1, 0, 2).reshape(128, 4 * 512) for h in range(G2)], axis=1)
        bi = np.concatenate([b1[e].reshape(KF, 128).T,
                             b2[e].reshape(KD, 128).T], axis=1)
        in_maps.append({
            "xTi": np.ascontiguousarray(xTi),
            "w1i": np.ascontiguousarray(w1p),
            "w2i": np.ascontiguousarray(w2p),
            "bi": np.ascontiguousarray(bi),
        })
    return in_maps, toks_per_core, chunks


def kernel(x, Wg, bg, W1, b1, W2, b2):
    from concourse.bass_utils import run_bass_kernel_spmd

    x = np.asarray(x, dtype=np.float32)
    n_tok = x.shape[0]

    # host gate in f64: the mathematically-true argmax
    logits = x.astype(np.float64) @ np.asarray(Wg, np.float64) + np.asarray(bg, np.float64)
    idx = logits.argmax(1)

    counts = np.bincount(idx, minlength=E)
    order = np.argsort(idx, kind="stable")
    starts = np.zeros(E + 1, np.int64)
    starts[1:] = np.cumsum(counts)

    C = max(int(counts.max()), 256)
    C = (C + 15) // 16 * 16

    W1 = np.asarray(W1, np.float32)
    W2 = np.asarray(W2, np.float32)
    b1 = np.asarray(b1, np.float32)
    b2 = np.asarray(b2, np.float32)

    in_maps, toks_per_core, chunks = _pack_inputs(x, W1, b1, W2, b2,
                                                  idx, order, starts, C)
    nc = _get_nc(C)
    res = run_bass_kernel_spmd(nc, in_maps, core_ids=list(range(N_CORES)))

    out = np.zeros((n_tok, D), np.float32)
    for e in range(E):
        toks = toks_per_core[e]
        ye = res.results[e]["yTi"].reshape(128, KD, C).transpose(2, 1, 0) \
            .reshape(C, D)
        out[toks] = ye[:len(toks)]
    return out


# revision 8
# speedup vs baseline: 1.4378x; 1.0038x over previous
"""MoE top-1 routing kernel for Trainium2 (8 NeuronCores, expert-parallel).

Model (E=8, D=512, F=2048, N=4096):
    logits = x @ Wg + bg; e = argmax(logits)
    y[i] = relu(x[i] @ W1[e] + b1[e]) @ W2[e] + b2[e]

Strategy:
- Host computes the gate (f64 matmul + argmax) and routes tokens; core e gets
  only expert e's tokens (padded to the max expert count C) + expert e's
  weights, and runs a dense 2-layer MLP in fp32r (full-rate fp32 matmul mode,
  ~2e-4 scale-relative error).
- All tensors are host-packed into SBUF-native [128, *] layouts so every DMA
  moves multi-KB contiguous runs per partition (one dma_start per piece).
- DMA pieces are issued in consumption order (x chunk 0, W1 by f-groups,
  W2 by fo-groups) so matmuls start ~5us in; stage-2 accumulation is emitted
  in W2-piece arrival order, interleaved across PSUM banks.
- A short dummy-matmul burst warms the PE clock (HAM) during the DMA head.
- Tokens ride the matmul free dim in chunks of <=512 columns (PSUM bank
  limit), >=256 wide where possible (fp32r full-rate threshold).
"""

import sys

sys.path.insert(0, "/opt/trn_rl_repo")

import numpy as np

E, D, F, N_CORES = 8, 512, 2048, 8
KD, KF = D // 128, F // 128  # 4, 16
G1, G2 = KF // 4, KF // 4    # w1 f-piece count, w2 fo-piece count (4 each)

_cache: dict = {}


def _build(C: int, chunks: list[tuple[int, int]]):
    import concourse.tile as tile
    import concourse.mybir as mybir
    from concourse import bacc

    f32, f32r = mybir.dt.float32, mybir.dt.float32r
    Relu = mybir.ActivationFunctionType.Relu

    nc = bacc.Bacc("TRN2", target_bir_lowering=False, debug=False)
    # packed layouts, all [128, *]:
    #   xTi[p, chunk_off + ko*cw + c] = x_e[c0+c, 128*ko+p]
    #   w1i[p, g*2048 + ko*512 + fi]  = W1_e[128*ko+p, 512*g+fi]
    #   w2i[p, h*2048 + j*512 + d]    = W2_e[128*(4h+j)+p, d]
    #   bi[p, f] = b1_e[128f+p] (f<16);  bi[p, 16+d] = b2_e[128d+p]
    #   yTi[p, d*C + c] = y_e[c, 128d+p]
    xTi = nc.dram_tensor("xTi", [128, KD * C], f32r, kind="ExternalInput").ap()
    w1i = nc.dram_tensor("w1i", [128, KD * F], f32r, kind="ExternalInput").ap()
    w2i = nc.dram_tensor("w2i", [128, KF * D], f32r, kind="ExternalInput").ap()
    bi = nc.dram_tensor("bi", [128, KF + KD], f32, kind="ExternalInput").ap()
    yTi = nc.dram_tensor("yTi", [128, KD * C], f32, kind="ExternalOutput").ap()
    y3 = yTi.rearrange("p (d c) -> p d c", c=C)

    with tile.TileContext(nc) as tc:
        with tc.tile_pool(name="wp", bufs=1) as wp, \
             tc.tile_pool(name="hp", bufs=1) as hp, \
             tc.tile_pool(name="yp", bufs=2) as yp, \
             tc.tile_pool(name="scr", bufs=1) as scr, \
             tc.tile_pool(name="pp", bufs=3, space="PSUM") as pp:

            # --- PE warm-up: dummy matmuls during the DMA head (HAM ramp).
            # f32 runs 4 cycles/row: N=128 -> ~427ns cold each, so 9 of them
            # cover the ~3.4us HAM window while delaying real matmuls <0.5us.
            wrm = scr.tile([128, 128], f32, name="wrm")
            nc.vector.memset(wrm[:], 0.0)
            wps = pp.tile([128, 128], f32, name="wps", tag="wps", bufs=1)
            for _ in range(9):
                nc.tensor.matmul(wps[:], wrm[:], wrm[:], start=True, stop=True)

            # --- DMA issue, consumption order, single engine (sync) ---
            bis = wp.tile([128, KF + KD], f32, name="bis")
            nc.sync.dma_start(bis[:], bi[:])

            w1t = wp.tile([128, KD * F], f32r, name="w1t")
            w2t = wp.tile([128, KF * D], f32r, name="w2t")
            xs = []
            off = 0
            for ci, (c0, c1) in enumerate(chunks):
                cw = c1 - c0
                xst = wp.tile([128, KD * cw], f32r, name=f"xs{ci}", tag=f"xs{ci}")
                xs.append((xst, off))
                off += KD * cw
            # Issue order = consumption order. Per-queue throughput only ramps
            # with several dma_starts outstanding (2KB max packet), so the
            # early pieces are small and issued back-to-back; w1 is split in
            # half-pieces (512KB) to keep supply granularity fine.
            def dma_piece(dst, src, lo, hi):
                nc.sync.dma_start(dst[:, lo:hi], src[:, lo:hi])

            xst0, o0 = xs[0]
            half = (xst0.shape[-1] // 2) // KD * KD if False else xst0.shape[-1] // 2
            nc.sync.dma_start(xst0[:, :half], xTi[:, o0:o0 + half])
            dma_piece(w1t, w1i, 0, 1024)
            nc.sync.dma_start(xst0[:, half:], xTi[:, o0 + half:o0 + xst0.shape[-1]])
            dma_piece(w1t, w1i, 1024, 2048)
            if len(xs) > 1:
                xst1, o1 = xs[1]
                nc.sync.dma_start(xst1[:], xTi[:, o1:o1 + xst1.shape[-1]])
            for g in range(1, G1):
                dma_piece(w1t, w1i, g * 2048, g * 2048 + 1024)
                dma_piece(w1t, w1i, g * 2048 + 1024, (g + 1) * 2048)
            for xst, o in xs[2:]:
                nc.sync.dma_start(xst[:], xTi[:, o:o + xst.shape[-1]])
            for h in range(G2):
                dma_piece(w2t, w2i, h * 2048, h * 2048 + 1024)
                dma_piece(w2t, w2i, h * 2048 + 1024, (h + 1) * 2048)

            # --- stage 1: h = relu(x @ W1 + b1), emitted in w1-piece order ---
            hs = {}  # (ci, f) -> tile
            for g in range(G1):
                for ci, (c0, c1) in enumerate(chunks):
                    cw = c1 - c0
                    xst = xs[ci][0]
                    for f in range(4 * g, 4 * g + 4):
                        p1 = pp.tile([128, cw], f32, name=f"p1_{ci}_{f}", tag="p1")
                        for ko in range(KD):
                            lhsT = w1t[:, g * 2048 + ko * 512 + (f % 4) * 128:
                                       g * 2048 + ko * 512 + (f % 4) * 128 + 128]
                            nc.tensor.matmul(p1[:], lhsT, xst[:, ko * cw:(ko + 1) * cw],
                                             start=(ko == 0), stop=(ko == KD - 1))
                        h = hp.tile([128, cw], f32r, name=f"h{ci}_{f}",
                                    tag=f"h{ci % 2}_{f}")
                        nc.scalar.activation(h[:], p1[:], Relu, bias=bis[:, f:f + 1])
                        hs[(ci, f)] = h

            # --- stage 2: y = h @ W2 + b2, emitted in w2-piece arrival order ---
            for ci, (c0, c1) in enumerate(chunks):
                cw = c1 - c0
                p2s = [pp.tile([128, cw], f32, name=f"p2_{ci}_{d}", tag=f"p2_{d}",
                               bufs=1) for d in range(KD)]
                for h2 in range(G2):
                    for d in range(KD):
                        for j in range(4):
                            fo = 4 * h2 + j
                            lhsT = w2t[:, h2 * 2048 + j * 512 + d * 128:
                                       h2 * 2048 + j * 512 + d * 128 + 128]
                            nc.tensor.matmul(p2s[d][:], lhsT, hs[(ci, fo)][:],
                                             start=(fo == 0), stop=(fo == KF - 1))
                ys = yp.tile([128, KD, cw], f32, name=f"ys{ci}", tag="ys")
                out_engs = [nc.gpsimd, nc.scalar, nc.gpsimd, nc.scalar]
                for d in range(KD):
                    nc.vector.tensor_scalar_add(ys[:, d, :], p2s[d][:],
                                                bis[:, KF + d:KF + d + 1])
                    out_engs[d].dma_start(y3[:, d, c0:c1], ys[:, d, :])
    nc.compile()
    return nc


def _plan_chunks(C: int) -> list[tuple[int, int]]:
    n = max(1, -(-C // 512))
    base, rem = divmod(C, n)
    out, pos = [], 0
    for i in range(n):
        w = base + (1 if i < rem else 0)
        out.append((pos, pos + w))
        pos += w
    return out


def _get_nc(C: int):
    if C not in _cache:
        _cache[C] = _build(C, _plan_chunks(C))
    return _cache[C]


def _pack_inputs(x, W1, b1, W2, b2, idx, order, starts, C):
    chunks = _plan_chunks(C)
    in_maps, toks_per_core = [], []
    for e in range(E):
        toks = order[starts[e]:starts[e + 1]]
        toks_per_core.append(toks)
        xe = np.zeros((C, D), np.float32)
        xe[:len(toks)] = x[toks]
        xeT = xe.T  # [D, C]
        xTi = np.concatenate(
            [xeT[:, c0:c1].reshape(KD, 128, c1 - c0).transpose(1, 0, 2)
             .reshape(128, KD * (c1 - c0)) for c0, c1 in chunks], axis=1)
        w1p = np.concatenate(
            [W1[e][:, 512 * g:512 * (g + 1)].reshape(KD, 128, 512)
             .transpose(1, 0, 2).reshape(128, KD * 512) for g in range(G1)], axis=1)
        w2p = np.concatenate(
            [W2[e][512 * h:512 * (h + 1), :].reshape(4, 128, 512)
             .transpose(1, 0, 2).reshape(128, 4 * 512) for h in range(G2)], axis=1)
        bi = np.concatenate([b1[e].reshape(KF, 128).T,
                             b2[e].reshape(KD, 128).T], axis=1)
        in_maps.append({
            "xTi": np.ascontiguousarray(xTi),
            "w1i": np.ascontiguousarray(w1p),
            "w2i": np.ascontiguousarray(w2p),
            "bi": np.ascontiguousarray(bi),
        })
    return in_maps, toks_per_core, chunks


def kernel(x, Wg, bg, W1, b1, W2, b2):
    from concourse.bass_utils import run_bass_kernel_spmd

    x = np.asarray(x, dtype=np.float32)
    n_tok = x.shape[0]

    # host gate in f64: the mathematically-true argmax
    logits = x.astype(np.float64) @ np.asarray(Wg, np.float64) + np.asarray(bg, np.float64)
    idx = logits.argmax(1)

    counts = np.bincount(idx, minlength=E)
    order = np.argsort(idx, kind="stable")
    starts = np.zeros(E + 1, np.int64)
    starts[1:] = np.cumsum(counts)

    C = max(int(counts.max()), 256)
    C = (C + 15) // 16 * 16

    W1 = np.asarray(W1, np.float32)
    W2 = np.asarray(W2, np.float32)
    b1 = np.asarray(b1, np.float32)
    b2 = np.asarray(b2, np.float32)

    in_maps, toks_per_core, chunks = _pack_inputs(x, W1, b1, W2, b2,
                                                  idx, order, starts, C)
    nc = _get_nc(C)
    res = run_bass_kernel_spmd(nc, in_maps, core_ids=list(range(N_CORES)))

    out = np.zeros((n_tok, D), np.float32)
    for e in range(E):
        toks = toks_per_core[e]
        ye = res.results[e]["yTi"].reshape(128, KD, C).transpose(2, 1, 0) \
            .reshape(C, D)
        out[toks] = ye[:len(toks)]
    return out
